# revision 17
# baseline (speedup 1.0000x reference)
"""Sliding-window causal GQA attention block (QKV proj + RoPE + SDPA + out proj)
on 8 Trainium2 NeuronCores.

Sharding: 8 cores = 2 batches x 4 sequence chunks of 512 tokens. Each core
computes the full attention-block output for its (batch, seq-chunk):
  - K/V projection for its chunk + 512-token halo (sliding window support)
  - Q projection for its 512 queries (all 16 heads) in transposed [d, s]
    layout; RoPE via rotate-half permutation matmul + element-wise mul/add.
  - attention runs on blocks of (kv-group, 128-query sub-chunk) with all 4
    query heads of the group sharing the 512 matmul columns; each block needs
    exactly 5 key tiles of 128 (vs 6 for 256-query blocks).
  - softmax denominators: the 5 masked-exp tiles are summed on the vector
    engine, then a single ones-vector matmul per block reduces over keys.
  - attention blocks are interleaved instruction-by-instruction with the
    Q-projection chains so the tensor engine stays dense while exp/mask run
    on the scalar/vector engines.
  - out-projection computed transposed (y^T = wo^T-tiles @ o^T); host
    transposes each core's bf16 slab back and casts to f32.

Weights/x are pre-packed on the host into per-partition-contiguous layouts so
every DMA moves 2-16 KB per partition line. Matmul operands are bf16, all
accumulation fp32 in PSUM.
"""
import numpy as np

import concourse.bacc as bacc
import concourse.mybir as mybir
import concourse.tile as tile
from concourse.bass_utils import run_bass_kernel_spmd

# Problem constants (hardcoded per contract)
B, S, E = 2, 2048, 2048
H, KV, D = 16, 4, 128
WIN = 512
THETA = 1e6
NCORES = 8
CH = 512          # seq chunk per core
P = 128
ECH = E // P      # 16 contraction chunks
F32 = mybir.dt.float32
BF16 = mybir.dt.bfloat16
SCALE = 1.0 / float(np.sqrt(np.float32(D)))

_CACHE = {}


def _build():
    nc = bacc.Bacc("TRN2", target_bir_lowering=False, debug=False,
                   num_devices=NCORES)

    # host-prepacked inputs: per-partition-contiguous layouts
    xt_own = nc.dram_tensor("xt_own", [P, ECH * CH], BF16, kind="ExternalInput")
    xt_halo = nc.dram_tensor("xt_halo", [P, ECH * CH], BF16,
                             kind="ExternalInput")
    wq_all = nc.dram_tensor("wq_all", [H, P, ECH * P], BF16, kind="ExternalInput")
    wk_all = nc.dram_tensor("wk_all", [KV, P, ECH * P], BF16, kind="ExternalInput")
    wv_all = nc.dram_tensor("wv_all", [P, ECH * KV * D], BF16, kind="ExternalInput")
    wo_all = nc.dram_tensor("wo_all", [ECH, P, H * P], BF16, kind="ExternalInput")
    cosw = nc.dram_tensor("cosw", [P, 2 * CH], F32, kind="ExternalInput")
    sinw = nc.dram_tensor("sinw", [P, 2 * CH], F32, kind="ExternalInput")
    masks = nc.dram_tensor("masks", [20, P, CH], BF16, kind="ExternalInput")
    perm = nc.dram_tensor("perm", [P, P], BF16, kind="ExternalInput")
    ones = nc.dram_tensor("ones", [1, P], BF16, kind="ExternalInput")
    yt = nc.dram_tensor("yt", [E, CH], BF16, kind="ExternalOutput")

    with tile.TileContext(nc) as tc:
        with (
            tc.tile_pool(name="res", bufs=1) as res,       # resident tensors
            tc.tile_pool(name="wvp", bufs=1) as wvp,       # resident wv
            tc.tile_pool(name="wkp", bufs=2) as wkp,       # streamed wk tiles
            tc.tile_pool(name="wqp", bufs=2) as wqp,       # streamed wq tiles
            tc.tile_pool(name="wop", bufs=2) as wop,       # streamed wo tiles
            tc.tile_pool(name="tmp", bufs=3) as tmp,       # transient compute
            tc.tile_pool(name="acc", bufs=2) as accp,      # pt-sum accumulators
            tc.tile_pool(name="pj", bufs=2, space="PSUM") as pj,
            tc.tile_pool(name="ps1", bufs=2, space="PSUM") as ps1,  # rot+scores
            tc.tile_pool(name="ps2", bufs=2, space="PSUM") as ps2,  # av
            tc.tile_pool(name="psd", bufs=2, space="PSUM") as psd,  # denom
        ):
            # ---- first K-head weights, then x pieces (sync queue) ----------
            x_halo = res.tile([P, ECH, CH], BF16, tag="xhalo")
            x_own = res.tile([P, ECH, CH], BF16, tag="xown")
            xh3 = xt_halo.ap().rearrange("p (eo s) -> p eo s", eo=ECH)
            xo3 = xt_own.ap().rearrange("p (eo s) -> p eo s", eo=ECH)
            wk_t = {}

            def load_wk(fk):
                wk_t[fk] = wkp.tile([P, ECH, P], BF16, tag="wk",
                                    name=f"wk_{fk}")
                nc.sync.dma_start(
                    wk_t[fk][:],
                    wk_all.ap().rearrange("h p c -> p h c")[:, fk, :]
                    .rearrange("p (eo c) -> p eo c", eo=ECH))

            load_wk(0)
            for lo, hi in ((0, 2), (2, 4), (4, 8), (8, 12), (12, 16)):
                nc.sync.dma_start(x_halo[:, lo:hi, :], xh3[:, lo:hi, :])
            for lo, hi in ((0, 4), (4, 8), (8, 16)):
                nc.sync.dma_start(x_own[:, lo:hi, :], xo3[:, lo:hi, :])

            # ---------------- constants (gpsimd queue) ----------------------
            cos_sb = res.tile([P, 2 * CH], F32, tag="cosw")
            sin_sb = res.tile([P, 2 * CH], F32, tag="sinw")
            nc.gpsimd.dma_start(cos_sb[:], cosw.ap())
            nc.gpsimd.dma_start(sin_sb[:], sinw.ap())
            perm_sb = res.tile([P, P], BF16, tag="perm")
            nc.gpsimd.dma_start(perm_sb[:], perm.ap())
            ones_sb = res.tile([P, 1], BF16, tag="ones")
            nc.gpsimd.dma_start(ones_sb[:], ones.ap().rearrange("o p -> p o"))
            ones_row = res.tile([1, P], BF16, tag="onesr")
            nc.gpsimd.dma_start(ones_row[:], ones.ap())

            # wv resident, [p, e_chunk, v_cols 512] (gpsimd queue)
            wv_sb = wvp.tile([P, ECH, KV * D], BF16, tag="wv")
            nc.gpsimd.dma_start(wv_sb[:], wv_all.ap().rearrange(
                "p (eo c) -> p eo c", eo=ECH))

            # ---------------- rope helper (split into prep + finish) --------
            def rope_prep(raw_ps):
                raw_sb = tmp.tile([P, CH], BF16, tag="qraw")
                nc.scalar.activation(out=raw_sb[:], in_=raw_ps[:],
                                     func=mybir.ActivationFunctionType.Copy)
                return raw_sb

            def rope_pe(raw_sb):
                rot_ps = ps1.tile([P, CH], F32, tag="sc")
                nc.tensor.matmul(rot_ps[:], perm_sb[:], raw_sb[:],
                                 start=True, stop=True)
                return rot_ps

            def rope_finish(dst, raw_sb, rot_ps, c0, split4=False):
                t1 = tmp.tile([P, CH], F32, tag="qraw")
                nc.gpsimd.tensor_mul(out=t1[:], in0=raw_sb[:],
                                     in1=cos_sb[:, c0:c0 + CH])
                t2 = tmp.tile([P, CH], F32, tag="qraw")
                nc.vector.tensor_mul(out=t2[:], in0=rot_ps[:],
                                     in1=sin_sb[:, c0:c0 + CH])
                if split4:
                    nc.gpsimd.tensor_add(
                        out=dst,
                        in0=t1[:].rearrange("p (a b) -> p a b", a=4),
                        in1=t2[:].rearrange("p (a b) -> p a b", a=4))
                else:
                    nc.gpsimd.tensor_add(out=dst, in0=t1[:], in1=t2[:])

            # ---------------- K/V projections (own + halo chains) -----------
            # K in transposed [d, s] layout; window = [halo 512 | own 512].
            k_sb = res.tile([P, KV, 2 * CH], BF16, tag="k")
            v_sb = res.tile([P, 8, KV * D], BF16, tag="v")

            pending_rope = None   # 1-chain delayed rope finish (no PE bubble)
            for fk in range(KV):
                if fk > 0:
                    load_wk(fk)
                for sh in range(2):
                    k_ps = pj.tile([P, CH], F32, tag="pj")
                    for e in range(ECH):
                        nc.tensor.matmul(
                            k_ps[:], wk_t[fk][:, e, :],
                            (x_halo if sh == 0 else x_own)[:, e, :],
                            start=(e == 0), stop=(e == ECH - 1))
                    if pending_rope is not None:
                        d_, rs_, rp_, c0_ = pending_rope
                        rope_finish(d_, rs_, rp_, c0_)
                    raw = rope_prep(k_ps)
                    rot = rope_pe(raw)
                    pending_rope = (k_sb[:, fk, sh * CH:(sh + 1) * CH],
                                    raw, rot, sh * CH)
            d_, rs_, rp_, c0_ = pending_rope
            rope_finish(d_, rs_, rp_, c0_)

            def x_win_tile(e, st):
                """lhsT [128 e-part, 128 pos-cols] for window pos-tile st."""
                if st < 4:
                    return x_halo[:, e, st * P:(st + 1) * P]
                return x_own[:, e, (st - 4) * P:(st - 3) * P]

            for st in range(8):
                v_ps = pj.tile([P, KV * D], F32, tag="pj")
                for e in range(ECH):
                    nc.tensor.matmul(v_ps[:], x_win_tile(e, st), wv_sb[:, e, :],
                                     start=(e == 0), stop=(e == ECH - 1))
                nc.scalar.activation(out=v_sb[:, st, :], in_=v_ps[:],
                                     func=mybir.ActivationFunctionType.Copy)

            # masks load on the scalar queue now; they only gate the
            # attention mask-muls and must not steal startup DMA bandwidth
            mask_sb = res.tile([P, 20, CH], BF16, tag="masks")
            for mi in range(20):
                nc.scalar.dma_start(mask_sb[:, mi, :], masks.ap()[mi])

            def k_tile(kvb, jt):
                return k_sb[:, kvb, jt * P:(jt + 1) * P]

            def v_tile(kvb, jt):
                return v_sb[:, jt, kvb * D:(kvb + 1) * D]

            # ------- Q projection interleaved with attention blocks ---------
            # q_sb free layout [kv, qsub, h4, qcol]: block (kv, qsub) holds the
            # same 128 queries for the 4 heads of kv-group kv.
            q_sb = res.tile([P, KV, 4, 4, P], BF16, tag="q")
            o_sb = res.tile([P, KV, 4, 4, P], BF16, tag="o")
            wq3 = wq_all.ap().rearrange("h p c -> p h c")

            def attn_block_ops(kvb, qs):
                """PE-op closures for one attention block, to be interleaved
                with q-chain matmuls; support ops are emitted alongside."""
                st = {"av": None, "ptsum": None, "pts": []}

                def mk_sc(r):
                    def f():
                        sc_ps = ps1.tile([P, CH], F32, tag="sc",
                                         name=f"sc_{kvb}_{qs}_{r}")
                        nc.tensor.matmul(sc_ps[:], k_tile(kvb, qs + r),
                                         q_sb[:, kvb, qs, :, :],
                                         start=True, stop=True)
                        pe = tmp.tile([P, CH], BF16, tag="pe")
                        nc.scalar.activation(
                            out=pe[:], in_=sc_ps[:],
                            func=mybir.ActivationFunctionType.Exp,
                            scale=SCALE)
                        pt = tmp.tile([P, CH], BF16, tag="pt",
                                      name=f"pt_{kvb}_{qs}_{r}")
                        nc.vector.tensor_mul(out=pt[:], in0=pe[:],
                                             in1=mask_sb[:, qs * 5 + r, :])
                        st["pts"].append(pt)
                        if st["ptsum"] is None:
                            st["ptsum"] = pt
                        else:
                            nxt = accp.tile([P, CH], BF16, tag="pts",
                                            name=f"pts_{kvb}_{qs}_{r}")
                            nc.vector.tensor_add(out=nxt[:], in0=st["ptsum"][:],
                                                 in1=pt[:])
                            st["ptsum"] = nxt
                    return f

                def mk_av(r):
                    def f():
                        if r == 0:
                            st["av"] = ps2.tile([P, CH], F32, tag="av",
                                                name=f"av_{kvb}_{qs}")
                        nc.tensor.matmul(st["av"][:], v_tile(kvb, qs + r),
                                         st["pts"][r][:],
                                         start=(r == 0), stop=(r == 4))
                    return f

                def mk_dn():
                    def f():
                        dn_ps = psd.tile([1, CH], F32, tag="dn")
                        nc.tensor.matmul(dn_ps[:], ones_sb[:],
                                         st["ptsum"][:], start=True, stop=True)
                        den = tmp.tile([1, CH], BF16, tag="den",
                                       name=f"den_{kvb}_{qs}")
                        nc.scalar.activation(
                            out=den[:], in_=dn_ps[:],
                            func=mybir.ActivationFunctionType.Copy)
                        st["den"] = den
                    return f

                def finisher():
                    # broadcast den across partitions with a contraction-1
                    # matmul (gpsimd custom-op library swaps are ~7us; avoid),
                    # then reciprocal + normalize on the vector engine
                    bc_ps = ps1.tile([P, CH], F32, tag="sc",
                                     name=f"bc_{kvb}_{qs}")
                    nc.tensor.matmul(bc_ps[:], ones_row[:], st["den"][:],
                                     start=True, stop=True)
                    rc = tmp.tile([P, CH], F32, tag="bc")
                    nc.vector.reciprocal_approx_fast(out=rc[:], in_=bc_ps[:])
                    nc.vector.tensor_mul(
                        out=o_sb[:, kvb, qs, :, :].rearrange(
                            "p a b -> p (a b)"),
                        in0=st["av"][:], in1=rc[:])

                # PE-op order: scores spread early, avs late so the
                # exp+mask chain latency hides behind q-chain matmuls
                return [mk_sc(0), mk_sc(1), mk_sc(2), mk_sc(3), mk_sc(4),
                        mk_av(0), mk_av(1), mk_av(2), mk_av(3), mk_av(4),
                        mk_dn()], finisher

            # after this many q-chain matmuls, emit the next block PE op
            POS = (2, 4, 6, 8, 10, 12, 13, 14, 15, 16, 16)

            blocks = [(kvb, qs) for kvb in range(KV) for qs in (3, 2, 1, 0)]

            def q_chain(fi, block_ops, prev_fin):
                kvb, h4 = fi // 4, fi % 4
                wq_t = wqp.tile([P, ECH, P], BF16, tag="wq", name=f"wq_{fi}")
                nc.sync.dma_start(
                    wq_t[:],
                    wq3[:, fi, :].rearrange("p (eo c) -> p eo c", eo=ECH))
                q_ps = pj.tile([P, CH], F32, tag="pj")
                bi = 0
                for e in range(ECH):
                    nc.tensor.matmul(q_ps[:], wq_t[:, e, :], x_own[:, e, :],
                                     start=(e == 0), stop=(e == ECH - 1))
                    if e == 0 and prev_fin is not None:
                        prev_fin()
                    while bi < len(block_ops) and POS[bi] <= e + 1:
                        block_ops[bi]()
                        bi += 1
                for op in block_ops[bi:]:
                    op()
                raw = rope_prep(q_ps)
                rot = rope_pe(raw)
                rope_finish(q_sb[:, kvb, :, h4, :], raw, rot, CH, split4=True)

            pend_fin = None
            for fi in range(H):
                if fi >= 4:
                    bkvb, bqs = blocks[fi - 4]
                    ops, fin = attn_block_ops(bkvb, bqs)
                    q_chain(fi, ops, pend_fin)
                    pend_fin = fin
                else:
                    q_chain(fi, [], None)
            # tail: last 4 attention blocks after all Q heads
            for bkvb, bqs in blocks[12:]:
                ops, fin = attn_block_ops(bkvb, bqs)
                ops[0]()
                if pend_fin is not None:
                    pend_fin()
                for op in ops[1:]:
                    op()
                pend_fin = fin
            pend_fin()

            # ------------- out projection, transposed: yt = sum_f woT @ oT ---
            wo3 = wo_all.ap().rearrange("h p c -> p h c")
            for et in range(ECH):
                wo_t = wop.tile([P, H, P], BF16, tag="wo", name=f"wo_{et}")
                nc.sync.dma_start(
                    wo_t[:],
                    wo3[:, et, :].rearrange("p (fo c) -> p fo c", fo=H))
                y_ps = pj.tile([P, CH], F32, tag="pj")
                for f in range(H):
                    nc.tensor.matmul(y_ps[:], wo_t[:, f, :],
                                     o_sb[:, f // 4, :, f % 4, :],
                                     start=(f == 0), stop=(f == H - 1))
                y_sb = tmp.tile([P, CH], BF16, tag="ysb")
                nc.scalar.activation(out=y_sb[:], in_=y_ps[:],
                                     func=mybir.ActivationFunctionType.Copy)
                nc.gpsimd.dma_start(yt.ap()[et * P:(et + 1) * P, :], y_sb[:])

    nc.compile()
    return nc


def _host_constants():
    import ml_dtypes
    inv_freq = (1.0 / (THETA ** (np.arange(0, D, 2, dtype=np.float32) / D))
                ).astype(np.float32)
    ang = np.arange(S, dtype=np.float32)[:, None] * inv_freq[None, :]
    emb = np.concatenate([ang, ang], axis=-1)          # [S, D]
    cos_t = np.ascontiguousarray(np.cos(emb).astype(np.float32).T)  # [D, S]
    sin_t = np.ascontiguousarray(np.sin(emb).astype(np.float32).T)

    pm = np.zeros((P, P), dtype=np.float32)            # rotate-half as lhsT
    a = np.arange(64)
    pm[a, a + 64] = 1.0
    pm[a + 64, a] = -1.0
    pm = pm.astype(ml_dtypes.bfloat16)

    onesv = np.ones((1, P), dtype=ml_dtypes.bfloat16)
    return cos_t, sin_t, pm, onesv


def _masks_for_chunk(chunk):
    """[20, 128, 512] bf16: mask[qs*5+r, j, :] for (qsub, r) blocks.

    Columns are 4 heads x 128 queries of sub-chunk qs; the mask depends only
    on the query position, so the four 128-col groups are equal."""
    import ml_dtypes
    m = np.zeros((20, P, CH), dtype=np.float32)
    s0 = chunk * CH
    for qs in range(4):
        qg = s0 + qs * P + np.arange(P)[None, :]       # [1, 128] query pos
        for r in range(5):
            jt = qs + r
            jg = s0 - WIN + jt * P + np.arange(P)[:, None]  # [128, 1] key pos
            dlt = qg - jg
            ok = ((dlt >= 0) & (dlt < WIN) & (jg >= 0)).astype(np.float32)
            m[qs * 5 + r] = np.tile(ok, (1, 4))
    return m.astype(ml_dtypes.bfloat16)


def _pack_pe(w, ncols):
    """[E, ncols] f32 -> [128, (E/128)*ncols] bf16, partition-contiguous."""
    import ml_dtypes
    return np.ascontiguousarray(
        w.reshape(ECH, P, ncols).transpose(1, 0, 2).reshape(P, ECH * ncols)
    ).astype(ml_dtypes.bfloat16)


def _pack_pe_x(xt_sl):
    """[E, 512] f32 -> [128, 16*512] bf16, partition-contiguous."""
    import ml_dtypes
    return np.ascontiguousarray(
        xt_sl.reshape(ECH, P, CH).transpose(1, 0, 2).reshape(P, ECH * CH)
    ).astype(ml_dtypes.bfloat16)


def _prepare_in_maps(x, w_qkv, w_o):
    cos_t, sin_t, pm, onesv = _host_constants()
    w_qkv = np.asarray(w_qkv, dtype=np.float32)
    w_o = np.asarray(w_o, dtype=np.float32)

    wq_all = np.stack([_pack_pe(w_qkv[:, f * P:(f + 1) * P], P)
                       for f in range(H)])             # [16, 128, 2048]
    KOFF = H * D
    VOFF = H * D + KV * D
    wk_all = np.stack([_pack_pe(w_qkv[:, KOFF + f * P:KOFF + (f + 1) * P], P)
                       for f in range(KV)])            # [4, 128, 2048]
    wv_all = _pack_pe(w_qkv[:, VOFF:VOFF + KV * D], KV * D)   # [128, 8192]
    wo_all = np.stack([_pack_pe(w_o[:, e * P:(e + 1) * P], P)
                       for e in range(ECH)])           # [16, 128, 2048]

    in_maps = []
    xts = [np.ascontiguousarray(np.asarray(x[b], dtype=np.float32).T)
           for b in range(B)]                          # [E, S] f32
    for c in range(NCORES):
        b, chunk = divmod(c, 4)
        s0 = chunk * CH
        lo = s0 - WIN
        xh = np.zeros((E, CH), dtype=np.float32)
        cw = np.zeros((P, 2 * CH), dtype=np.float32)
        sw = np.zeros((P, 2 * CH), dtype=np.float32)
        cw[:, CH:] = cos_t[:, s0:s0 + CH]
        sw[:, CH:] = sin_t[:, s0:s0 + CH]
        if lo >= 0:
            xh[:] = xts[b][:, lo:s0]
            cw[:, 0:CH] = cos_t[:, lo:s0]
            sw[:, 0:CH] = sin_t[:, lo:s0]
        im = {
            "xt_own": _pack_pe_x(xts[b][:, s0:s0 + CH]),
            "xt_halo": _pack_pe_x(xh),
            "wq_all": wq_all,
            "wk_all": wk_all,
            "wv_all": wv_all,
            "wo_all": wo_all,
            "cosw": cw,
            "sinw": sw,
            "masks": _masks_for_chunk(chunk),
            "perm": pm,
            "ones": onesv,
        }
        in_maps.append(im)
    return in_maps


def _install_ntff_shim():
    """bass_utils wants antenv.axon_hooks for trace=True under axon; this
    environment lacks that module, so synthesize it from the boot helper."""
    import sys
    import types
    if "antenv.axon_hooks" in sys.modules:
        return
    try:
        from trn_agent_boot.trn_boot import _ntff_profile_via_ctypes
        hook = _ntff_profile_via_ctypes("/opt/axon/libaxon_pjrt.so")
    except Exception:
        hook = None
    mod = types.ModuleType("antenv.axon_hooks")
    mod.get_axon_ntff_profile_hook = lambda: hook
    mod.set_axon_ntff_profile_hook = lambda h: None
    sys.modules["antenv.axon_hooks"] = mod


def run(x, w_qkv, w_o, trace=False):
    if "nc" not in _CACHE:
        _CACHE["nc"] = _build()
    nc = _CACHE["nc"]
    in_maps = _prepare_in_maps(np.asarray(x), np.asarray(w_qkv),
                               np.asarray(w_o))
    if trace:
        _install_ntff_shim()
    try:
        res = run_bass_kernel_spmd(nc, in_maps, list(range(NCORES)),
                                   trace=trace)
    except Exception:
        if not trace:
            raise
        res = run_bass_kernel_spmd(nc, in_maps, list(range(NCORES)),
                                   trace=False)
    y = np.empty((B, S, E), dtype=np.float32)
    for c in range(NCORES):
        b, chunk = divmod(c, 4)
        y[b, chunk * CH:(chunk + 1) * CH, :] = \
            res.results[c]["yt"].astype(np.float32).T
    return y, res


def kernel(x, w_qkv, w_o):
    y, _ = run(x, w_qkv, w_o, trace=False)
    return y


# revision 18
# speedup vs baseline: 1.1839x; 1.1839x over previous
"""Sliding-window causal GQA attention block (QKV proj + RoPE + SDPA + out proj)
on 8 Trainium2 NeuronCores.

Sharding: 8 cores = 2 batches x 4 sequence chunks of 512 tokens. Each core
computes the full attention-block output for its (batch, seq-chunk):
  - K/V projection for its chunk + 512-token halo (sliding window support)
  - Q projection for its 512 queries (all 16 heads) in transposed [d, s]
    layout; RoPE via rotate-half permutation matmul + element-wise mul/add.
  - attention runs on blocks of (kv-group, 128-query sub-chunk) with all 4
    query heads of the group sharing the 512 matmul columns; each block needs
    exactly 5 key tiles of 128 (vs 6 for 256-query blocks).
  - softmax denominators: the 5 masked-exp tiles are summed on the vector
    engine, then a single ones-vector matmul per block reduces over keys.
  - attention blocks are interleaved instruction-by-instruction with the
    Q-projection chains so the tensor engine stays dense while exp/mask run
    on the scalar/vector engines.
  - out-projection computed transposed (y^T = wo^T-tiles @ o^T); host
    transposes each core's bf16 slab back and casts to f32.

Weights/x are pre-packed on the host into per-partition-contiguous layouts so
every DMA moves 2-16 KB per partition line. Matmul operands are bf16, all
accumulation fp32 in PSUM.
"""
import numpy as np

import concourse.bacc as bacc
import concourse.mybir as mybir
import concourse.tile as tile
from concourse.bass_utils import run_bass_kernel_spmd

# Problem constants (hardcoded per contract)
B, S, E = 2, 2048, 2048
H, KV, D = 16, 4, 128
WIN = 512
THETA = 1e6
NCORES = 8
CH = 512          # seq chunk per core
P = 128
ECH = E // P      # 16 contraction chunks
F32 = mybir.dt.float32
BF16 = mybir.dt.bfloat16
SCALE = 1.0 / float(np.sqrt(np.float32(D)))

_CACHE = {}


def _build():
    nc = bacc.Bacc("TRN2", target_bir_lowering=False, debug=False,
                   num_devices=NCORES)

    # host-prepacked inputs: per-partition-contiguous layouts
    xt_own = nc.dram_tensor("xt_own", [P, ECH * CH], BF16, kind="ExternalInput")
    xt_halo = nc.dram_tensor("xt_halo", [P, ECH * CH], BF16,
                             kind="ExternalInput")
    wq_all = nc.dram_tensor("wq_all", [H, P, ECH * P], BF16, kind="ExternalInput")
    wk_all = nc.dram_tensor("wk_all", [KV, P, ECH * P], BF16, kind="ExternalInput")
    wv_all = nc.dram_tensor("wv_all", [P, ECH * KV * D], BF16, kind="ExternalInput")
    wo_all = nc.dram_tensor("wo_all", [ECH, P, H * P], BF16, kind="ExternalInput")
    cosw = nc.dram_tensor("cosw", [P, 2 * CH], F32, kind="ExternalInput")
    sinw = nc.dram_tensor("sinw", [P, 2 * CH], F32, kind="ExternalInput")
    masks = nc.dram_tensor("masks", [20, P, CH], BF16, kind="ExternalInput")
    perm = nc.dram_tensor("perm", [P, P], BF16, kind="ExternalInput")
    ones = nc.dram_tensor("ones", [1, P], BF16, kind="ExternalInput")
    yt = nc.dram_tensor("yt", [E, CH], BF16, kind="ExternalOutput")

    with tile.TileContext(nc) as tc:
        with (
            tc.tile_pool(name="res", bufs=1) as res,       # resident tensors
            tc.tile_pool(name="wvp", bufs=1) as wvp,       # resident wv
            tc.tile_pool(name="wkp", bufs=2) as wkp,       # streamed wk tiles
            tc.tile_pool(name="wqp", bufs=2) as wqp,       # streamed wq tiles
            tc.tile_pool(name="wop", bufs=2) as wop,       # streamed wo tiles
            tc.tile_pool(name="tmp", bufs=3) as tmp,       # transient compute
            tc.tile_pool(name="acc", bufs=2) as accp,      # pt-sum accumulators
            tc.tile_pool(name="pj", bufs=2, space="PSUM") as pj,
            tc.tile_pool(name="ps1", bufs=2, space="PSUM") as ps1,  # rot+scores
            tc.tile_pool(name="ps2", bufs=2, space="PSUM") as ps2,  # av
            tc.tile_pool(name="psd", bufs=2, space="PSUM") as psd,  # denom
        ):
            # ---- first K-head weights, then x pieces (sync queue) ----------
            x_halo = res.tile([P, ECH, CH], BF16, tag="xhalo")
            x_own = res.tile([P, ECH, CH], BF16, tag="xown")
            xh3 = xt_halo.ap().rearrange("p (eo s) -> p eo s", eo=ECH)
            xo3 = xt_own.ap().rearrange("p (eo s) -> p eo s", eo=ECH)
            wk_t = {}

            def load_wk(fk):
                wk_t[fk] = wkp.tile([P, ECH, P], BF16, tag="wk",
                                    name=f"wk_{fk}")
                nc.sync.dma_start(
                    wk_t[fk][:],
                    wk_all.ap().rearrange("h p c -> p h c")[:, fk, :]
                    .rearrange("p (eo c) -> p eo c", eo=ECH))

            load_wk(0)
            for lo, hi in ((0, 2), (2, 4), (4, 8), (8, 12), (12, 16)):
                nc.sync.dma_start(x_halo[:, lo:hi, :], xh3[:, lo:hi, :])
            for lo, hi in ((0, 4), (4, 8), (8, 16)):
                nc.sync.dma_start(x_own[:, lo:hi, :], xo3[:, lo:hi, :])

            # ---------------- constants (gpsimd queue) ----------------------
            perm_sb = res.tile([P, P], BF16, tag="perm")
            nc.gpsimd.dma_start(perm_sb[:], perm.ap())
            cos_sb = res.tile([P, 2 * CH], F32, tag="cosw")
            sin_sb = res.tile([P, 2 * CH], F32, tag="sinw")
            nc.gpsimd.dma_start(cos_sb[:], cosw.ap())
            nc.gpsimd.dma_start(sin_sb[:], sinw.ap())
            ones_sb = res.tile([P, 1], BF16, tag="ones")
            nc.gpsimd.dma_start(ones_sb[:], ones.ap().rearrange("o p -> p o"))
            ones_row = res.tile([1, P], BF16, tag="onesr")
            nc.gpsimd.dma_start(ones_row[:], ones.ap())

            # wv resident, [p, e_chunk, v_cols 512] (gpsimd queue)
            wv_sb = wvp.tile([P, ECH, KV * D], BF16, tag="wv")
            nc.gpsimd.dma_start(wv_sb[:], wv_all.ap().rearrange(
                "p (eo c) -> p eo c", eo=ECH))

            # ---------------- rope helper (split into prep + finish) --------
            def rope_prep(raw_ps):
                raw_sb = tmp.tile([P, CH], BF16, tag="qraw")
                nc.scalar.activation(out=raw_sb[:], in_=raw_ps[:],
                                     func=mybir.ActivationFunctionType.Copy)
                return raw_sb

            def rope_pe(raw_sb):
                rot_ps = ps1.tile([P, CH], F32, tag="sc")
                nc.tensor.matmul(rot_ps[:], perm_sb[:], raw_sb[:],
                                 start=True, stop=True)
                return rot_ps

            def rope_finish(dst, raw_sb, rot_ps, c0, split4=False):
                t1 = tmp.tile([P, CH], F32, tag="qraw")
                nc.gpsimd.tensor_mul(out=t1[:], in0=raw_sb[:],
                                     in1=cos_sb[:, c0:c0 + CH])
                t2 = tmp.tile([P, CH], F32, tag="qraw")
                nc.vector.tensor_mul(out=t2[:], in0=rot_ps[:],
                                     in1=sin_sb[:, c0:c0 + CH])
                if split4:
                    nc.gpsimd.tensor_add(
                        out=dst,
                        in0=t1[:].rearrange("p (a b) -> p a b", a=4),
                        in1=t2[:].rearrange("p (a b) -> p a b", a=4))
                else:
                    nc.gpsimd.tensor_add(out=dst, in0=t1[:], in1=t2[:])

            # ---- PE warm-up: dummy matmuls on perm while x streams in ------
            for wi in range(100):
                wu_ps = pj.tile([P, P], F32, tag="pj", name=f"wu_{wi}")
                nc.tensor.matmul(wu_ps[:], perm_sb[:], perm_sb[:],
                                 start=True, stop=True)

            # ---------------- K/V projections (own + halo chains) -----------
            # K in transposed [d, s] layout; window = [halo 512 | own 512].
            k_sb = res.tile([P, KV, 2 * CH], BF16, tag="k")
            v_sb = res.tile([P, 8, KV * D], BF16, tag="v")

            pending_rope = None   # 1-chain delayed rope finish (no PE bubble)
            for fk in range(KV):
                if fk > 0:
                    load_wk(fk)
                for sh in range(2):
                    k_ps = pj.tile([P, CH], F32, tag="pj")
                    for e in range(ECH):
                        nc.tensor.matmul(
                            k_ps[:], wk_t[fk][:, e, :],
                            (x_halo if sh == 0 else x_own)[:, e, :],
                            start=(e == 0), stop=(e == ECH - 1))
                    if pending_rope is not None:
                        d_, rs_, rp_, c0_ = pending_rope
                        rope_finish(d_, rs_, rp_, c0_)
                    raw = rope_prep(k_ps)
                    rot = rope_pe(raw)
                    pending_rope = (k_sb[:, fk, sh * CH:(sh + 1) * CH],
                                    raw, rot, sh * CH)
            d_, rs_, rp_, c0_ = pending_rope
            rope_finish(d_, rs_, rp_, c0_)

            def x_win_tile(e, st):
                """lhsT [128 e-part, 128 pos-cols] for window pos-tile st."""
                if st < 4:
                    return x_halo[:, e, st * P:(st + 1) * P]
                return x_own[:, e, (st - 4) * P:(st - 3) * P]

            for st in range(8):
                v_ps = pj.tile([P, KV * D], F32, tag="pj")
                for e in range(ECH):
                    nc.tensor.matmul(v_ps[:], x_win_tile(e, st), wv_sb[:, e, :],
                                     start=(e == 0), stop=(e == ECH - 1))
                nc.scalar.activation(out=v_sb[:, st, :], in_=v_ps[:],
                                     func=mybir.ActivationFunctionType.Copy)

            # masks load on the scalar queue now; they only gate the
            # attention mask-muls and must not steal startup DMA bandwidth
            mask_sb = res.tile([P, 20, CH], BF16, tag="masks")
            for mi in range(20):
                nc.scalar.dma_start(mask_sb[:, mi, :], masks.ap()[mi])

            def k_tile(kvb, jt):
                return k_sb[:, kvb, jt * P:(jt + 1) * P]

            def v_tile(kvb, jt):
                return v_sb[:, jt, kvb * D:(kvb + 1) * D]

            # ------- Q projection interleaved with attention blocks ---------
            # q_sb free layout [kv, qsub, h4, qcol]: block (kv, qsub) holds the
            # same 128 queries for the 4 heads of kv-group kv.
            q_sb = res.tile([P, KV, 4, 4, P], BF16, tag="q")
            o_sb = res.tile([P, KV, 4, 4, P], BF16, tag="o")
            wq3 = wq_all.ap().rearrange("h p c -> p h c")

            def attn_block_ops(kvb, qs):
                """PE-op closures for one attention block, to be interleaved
                with q-chain matmuls; support ops are emitted alongside."""
                st = {"av": None, "ptsum": None, "pts": []}

                def mk_sc(r):
                    def f():
                        sc_ps = ps1.tile([P, CH], F32, tag="sc",
                                         name=f"sc_{kvb}_{qs}_{r}")
                        nc.tensor.matmul(sc_ps[:], k_tile(kvb, qs + r),
                                         q_sb[:, kvb, qs, :, :],
                                         start=True, stop=True)
                        pe = tmp.tile([P, CH], BF16, tag="pe")
                        nc.scalar.activation(
                            out=pe[:], in_=sc_ps[:],
                            func=mybir.ActivationFunctionType.Exp,
                            scale=SCALE)
                        pt = tmp.tile([P, CH], BF16, tag="pt",
                                      name=f"pt_{kvb}_{qs}_{r}")
                        nc.vector.tensor_mul(out=pt[:], in0=pe[:],
                                             in1=mask_sb[:, qs * 5 + r, :])
                        st["pts"].append(pt)
                        if st["ptsum"] is None:
                            st["ptsum"] = pt
                        else:
                            nxt = accp.tile([P, CH], BF16, tag="pts",
                                            name=f"pts_{kvb}_{qs}_{r}")
                            nc.vector.tensor_add(out=nxt[:], in0=st["ptsum"][:],
                                                 in1=pt[:])
                            st["ptsum"] = nxt
                    return f

                def mk_av(r):
                    def f():
                        if r == 0:
                            st["av"] = ps2.tile([P, CH], F32, tag="av",
                                                name=f"av_{kvb}_{qs}")
                        nc.tensor.matmul(st["av"][:], v_tile(kvb, qs + r),
                                         st["pts"][r][:],
                                         start=(r == 0), stop=(r == 4))
                    return f

                def mk_dn():
                    def f():
                        dn_ps = psd.tile([1, CH], F32, tag="dn")
                        nc.tensor.matmul(dn_ps[:], ones_sb[:],
                                         st["ptsum"][:], start=True, stop=True)
                        den = tmp.tile([1, CH], BF16, tag="den",
                                       name=f"den_{kvb}_{qs}")
                        nc.scalar.activation(
                            out=den[:], in_=dn_ps[:],
                            func=mybir.ActivationFunctionType.Copy)
                        st["den"] = den
                    return f

                def finisher():
                    # broadcast den across partitions with a contraction-1
                    # matmul (gpsimd custom-op library swaps are ~7us; avoid),
                    # then reciprocal + normalize on the vector engine
                    bc_ps = ps1.tile([P, CH], F32, tag="sc",
                                     name=f"bc_{kvb}_{qs}")
                    nc.tensor.matmul(bc_ps[:], ones_row[:], st["den"][:],
                                     start=True, stop=True)
                    rc = tmp.tile([P, CH], F32, tag="bc")
                    nc.vector.reciprocal_approx_fast(out=rc[:], in_=bc_ps[:])
                    nc.vector.tensor_mul(
                        out=o_sb[:, kvb, :, qs, :],
                        in0=st["av"][:].rearrange("p (a b) -> p a b", a=4),
                        in1=rc[:].rearrange("p (a b) -> p a b", a=4))

                # PE-op order: scores spread early, avs late so the
                # exp+mask chain latency hides behind q-chain matmuls
                return [mk_sc(0), mk_sc(1), mk_sc(2), mk_sc(3), mk_sc(4),
                        mk_av(0), mk_av(1), mk_av(2), mk_av(3), mk_av(4),
                        mk_dn()], finisher

            # after this many q-chain matmuls, emit the next block PE op
            POS = (2, 4, 6, 8, 10, 12, 13, 14, 15, 16, 16)

            blocks = [(kvb, qs) for kvb in range(KV) for qs in (3, 2, 1, 0)]

            def q_chain(fi, block_ops, prev_fin):
                kvb, h4 = fi // 4, fi % 4
                wq_t = wqp.tile([P, ECH, P], BF16, tag="wq", name=f"wq_{fi}")
                nc.sync.dma_start(
                    wq_t[:],
                    wq3[:, fi, :].rearrange("p (eo c) -> p eo c", eo=ECH))
                q_ps = pj.tile([P, CH], F32, tag="pj")
                bi = 0
                for e in range(ECH):
                    nc.tensor.matmul(q_ps[:], wq_t[:, e, :], x_own[:, e, :],
                                     start=(e == 0), stop=(e == ECH - 1))
                    if e == 0 and prev_fin is not None:
                        prev_fin()
                    while bi < len(block_ops) and POS[bi] <= e + 1:
                        block_ops[bi]()
                        bi += 1
                for op in block_ops[bi:]:
                    op()
                raw = rope_prep(q_ps)
                rot = rope_pe(raw)
                rope_finish(q_sb[:, kvb, :, h4, :], raw, rot, CH, split4=True)

            pend_fin = None
            for fi in range(H):
                if fi >= 4:
                    bkvb, bqs = blocks[fi - 4]
                    ops, fin = attn_block_ops(bkvb, bqs)
                    q_chain(fi, ops, pend_fin)
                    pend_fin = fin
                else:
                    q_chain(fi, [], None)
            # tail: last 4 attention blocks after all Q heads
            for bkvb, bqs in blocks[12:]:
                ops, fin = attn_block_ops(bkvb, bqs)
                ops[0]()
                if pend_fin is not None:
                    pend_fin()
                for op in ops[1:]:
                    op()
                pend_fin = fin
            pend_fin()

            # ------------- out projection, transposed: yt = sum_f woT @ oT ---
            wo3 = wo_all.ap().rearrange("h p c -> p h c")
            for et in range(ECH):
                wo_t = wop.tile([P, H, P], BF16, tag="wo", name=f"wo_{et}")
                nc.sync.dma_start(
                    wo_t[:],
                    wo3[:, et, :].rearrange("p (fo c) -> p fo c", fo=H))
                y_ps = pj.tile([P, CH], F32, tag="pj")
                for f in range(H):
                    nc.tensor.matmul(
                        y_ps[:], wo_t[:, f, :],
                        o_sb[:, f // 4, f % 4, :, :].rearrange(
                            "p a b -> p (a b)"),
                        start=(f == 0), stop=(f == H - 1))
                y_sb = tmp.tile([P, CH], BF16, tag="ysb")
                nc.scalar.activation(out=y_sb[:], in_=y_ps[:],
                                     func=mybir.ActivationFunctionType.Copy)
                nc.gpsimd.dma_start(yt.ap()[et * P:(et + 1) * P, :], y_sb[:])

    nc.compile()
    return nc


def _host_constants():
    import ml_dtypes
    inv_freq = (1.0 / (THETA ** (np.arange(0, D, 2, dtype=np.float32) / D))
                ).astype(np.float32)
    ang = np.arange(S, dtype=np.float32)[:, None] * inv_freq[None, :]
    emb = np.concatenate([ang, ang], axis=-1)          # [S, D]
    cos_t = np.ascontiguousarray(np.cos(emb).astype(np.float32).T)  # [D, S]
    sin_t = np.ascontiguousarray(np.sin(emb).astype(np.float32).T)

    pm = np.zeros((P, P), dtype=np.float32)            # rotate-half as lhsT
    a = np.arange(64)
    pm[a, a + 64] = 1.0
    pm[a + 64, a] = -1.0
    pm = pm.astype(ml_dtypes.bfloat16)

    onesv = np.ones((1, P), dtype=ml_dtypes.bfloat16)
    return cos_t, sin_t, pm, onesv


def _masks_for_chunk(chunk):
    """[20, 128, 512] bf16: mask[qs*5+r, j, :] for (qsub, r) blocks.

    Columns are 4 heads x 128 queries of sub-chunk qs; the mask depends only
    on the query position, so the four 128-col groups are equal."""
    import ml_dtypes
    m = np.zeros((20, P, CH), dtype=np.float32)
    s0 = chunk * CH
    for qs in range(4):
        qg = s0 + qs * P + np.arange(P)[None, :]       # [1, 128] query pos
        for r in range(5):
            jt = qs + r
            jg = s0 - WIN + jt * P + np.arange(P)[:, None]  # [128, 1] key pos
            dlt = qg - jg
            ok = ((dlt >= 0) & (dlt < WIN) & (jg >= 0)).astype(np.float32)
            m[qs * 5 + r] = np.tile(ok, (1, 4))
    return m.astype(ml_dtypes.bfloat16)


def _pack_pe(w, ncols):
    """[E, ncols] f32 -> [128, (E/128)*ncols] bf16, partition-contiguous."""
    import ml_dtypes
    return np.ascontiguousarray(
        w.reshape(ECH, P, ncols).transpose(1, 0, 2).reshape(P, ECH * ncols)
    ).astype(ml_dtypes.bfloat16)


def _pack_pe_x(xt_sl):
    """[E, 512] f32 -> [128, 16*512] bf16, partition-contiguous."""
    import ml_dtypes
    return np.ascontiguousarray(
        xt_sl.reshape(ECH, P, CH).transpose(1, 0, 2).reshape(P, ECH * CH)
    ).astype(ml_dtypes.bfloat16)


def _prepare_in_maps(x, w_qkv, w_o):
    cos_t, sin_t, pm, onesv = _host_constants()
    w_qkv = np.asarray(w_qkv, dtype=np.float32)
    w_o = np.asarray(w_o, dtype=np.float32)

    wq_all = np.stack([_pack_pe(w_qkv[:, f * P:(f + 1) * P], P)
                       for f in range(H)])             # [16, 128, 2048]
    KOFF = H * D
    VOFF = H * D + KV * D
    wk_all = np.stack([_pack_pe(w_qkv[:, KOFF + f * P:KOFF + (f + 1) * P], P)
                       for f in range(KV)])            # [4, 128, 2048]
    wv_all = _pack_pe(w_qkv[:, VOFF:VOFF + KV * D], KV * D)   # [128, 8192]
    wo_all = np.stack([_pack_pe(w_o[:, e * P:(e + 1) * P], P)
                       for e in range(ECH)])           # [16, 128, 2048]

    in_maps = []
    xts = [np.ascontiguousarray(np.asarray(x[b], dtype=np.float32).T)
           for b in range(B)]                          # [E, S] f32
    for c in range(NCORES):
        b, chunk = divmod(c, 4)
        s0 = chunk * CH
        lo = s0 - WIN
        xh = np.zeros((E, CH), dtype=np.float32)
        cw = np.zeros((P, 2 * CH), dtype=np.float32)
        sw = np.zeros((P, 2 * CH), dtype=np.float32)
        cw[:, CH:] = cos_t[:, s0:s0 + CH]
        sw[:, CH:] = sin_t[:, s0:s0 + CH]
        if lo >= 0:
            xh[:] = xts[b][:, lo:s0]
            cw[:, 0:CH] = cos_t[:, lo:s0]
            sw[:, 0:CH] = sin_t[:, lo:s0]
        im = {
            "xt_own": _pack_pe_x(xts[b][:, s0:s0 + CH]),
            "xt_halo": _pack_pe_x(xh),
            "wq_all": wq_all,
            "wk_all": wk_all,
            "wv_all": wv_all,
            "wo_all": wo_all,
            "cosw": cw,
            "sinw": sw,
            "masks": _masks_for_chunk(chunk),
            "perm": pm,
            "ones": onesv,
        }
        in_maps.append(im)
    return in_maps


def _install_ntff_shim():
    """bass_utils wants antenv.axon_hooks for trace=True under axon; this
    environment lacks that module, so synthesize it from the boot helper."""
    import sys
    import types
    if "antenv.axon_hooks" in sys.modules:
        return
    try:
        from trn_agent_boot.trn_boot import _ntff_profile_via_ctypes
        hook = _ntff_profile_via_ctypes("/opt/axon/libaxon_pjrt.so")
    except Exception:
        hook = None
    mod = types.ModuleType("antenv.axon_hooks")
    mod.get_axon_ntff_profile_hook = lambda: hook
    mod.set_axon_ntff_profile_hook = lambda h: None
    sys.modules["antenv.axon_hooks"] = mod


def run(x, w_qkv, w_o, trace=False):
    if "nc" not in _CACHE:
        _CACHE["nc"] = _build()
    nc = _CACHE["nc"]
    in_maps = _prepare_in_maps(np.asarray(x), np.asarray(w_qkv),
                               np.asarray(w_o))
    if trace:
        _install_ntff_shim()
    try:
        res = run_bass_kernel_spmd(nc, in_maps, list(range(NCORES)),
                                   trace=trace)
    except Exception:
        if not trace:
            raise
        res = run_bass_kernel_spmd(nc, in_maps, list(range(NCORES)),
                                   trace=False)
    y = np.empty((B, S, E), dtype=np.float32)
    for c in range(NCORES):
        b, chunk = divmod(c, 4)
        y[b, chunk * CH:(chunk + 1) * CH, :] = \
            res.results[c]["yt"].astype(np.float32).T
    return y, res


def kernel(x, w_qkv, w_o):
    y, _ = run(x, w_qkv, w_o, trace=False)
    return y
